# revision 1
# baseline (speedup 1.0000x reference)
"""Trainium2 Bass kernel for the NEUROPULS photonic-mesh transfer matrix.

Key insight: the reference's crossing layers are discarded, so every MMI layer
mixes the same fixed row pairs (2k, 2k+1) and every heater is diagonal.  The
full 512x512 transfer matrix is therefore block-diagonal with 256 independent
2x2 complex blocks:

    G_k = E_out(k) . Prod_{i=255..0} S_i(k) . E_in(k),
    S_i = B(2i+1) . diag(e^{i phi}) . B(2i),   B = [[t, i kappa], [i kappa, t]]

Each of the 8 cores computes the partial chain product of its 32 iterations
for all 256 pairs (a 5-level binary tree of batched 2x2 complex multiplies,
mapped onto TensorE matmuls + VectorE elementwise products).  The host then
multiplies the 8 per-core 2x2 partials, applies the diagonal phase factors,
and scatters the blocks into the zero matrix.
"""

import sys

sys.path.insert(0, "/opt/trn_rl_repo")

import numpy as np

N = 512
NPAIR = 256
NCORE = 8
CH = 32  # iterations per core
TWO_PI = 2.0 * np.pi

# ---------------------------------------------------------------------------
# combine-tree constants
# comp order: [00re,00im,01re,01im,10re,10im,11re,11im]
# C = A @ B decomposed as   c8 = W32 . ((PX.a8) * (PY.b8))
# ---------------------------------------------------------------------------


def _cidx(r, s, rho):
    return (r * 2 + s) * 2 + rho


def _build_consts():
    PX = np.zeros((32, 8), np.float32)
    PY = np.zeros((32, 8), np.float32)
    W32 = np.zeros((8, 32), np.float32)
    for r in range(2):
        for s in range(2):
            for rho in range(2):
                c8 = _cidx(r, s, rho)
                for m in range(2):
                    for part in range(2):
                        tau = c8 * 4 + m * 2 + part
                        if rho == 0:
                            aA = _cidx(r, m, part)
                            aB = _cidx(m, s, part)
                            sg = 1.0 if part == 0 else -1.0
                        else:
                            aA = _cidx(r, m, part)
                            aB = _cidx(m, s, 1 - part)
                            sg = 1.0
                        PX[tau, aA] = 1.0
                        PY[tau, aB] = 1.0
                        W32[c8, tau] = sg
    # W16: S' comps from PR rows, tau = taut*4+cq, taut in [ca,cb,sa,sb],
    # cq in [tt,kk,tk,kt]; primed trig means S' = -S (cancels over 32 steps).
    CA, CB, SA, SB = 0, 1, 2, 3
    TT, KK, TK, KT = 0, 1, 2, 3
    W16 = np.zeros((8, 16), np.float32)
    terms = {
        _cidx(0, 0, 0): [(CA, TT, +1), (CB, KK, -1)],
        _cidx(0, 0, 1): [(SA, TT, +1), (SB, KK, -1)],
        _cidx(0, 1, 0): [(SA, TK, -1), (SB, KT, -1)],
        _cidx(0, 1, 1): [(CA, TK, +1), (CB, KT, +1)],
        _cidx(1, 0, 0): [(SA, KT, -1), (SB, TK, -1)],
        _cidx(1, 0, 1): [(CA, KT, +1), (CB, TK, +1)],
        _cidx(1, 1, 0): [(CA, KK, -1), (CB, TT, +1)],
        _cidx(1, 1, 1): [(SA, KK, -1), (SB, TT, +1)],
    }
    for c8, tl in terms.items():
        for taut, cq, sg in tl:
            W16[c8, taut * 4 + cq] = sg
    return PX, PY, W32, W16


def _build_stationaries():
    """Pack all matmul stationary (lhsT) matrices into one [128, 712] array.

    lhsT[k_in, m_out] = A[m_out, k_in].
    f-offsets: 0 L1X[64,128], 128 L1Y[64,128], 256 L23X[128,128],
    384 L23Y[128,128], 512 L4[128,128], 640 L5[64,64], 704 LF[32,8].
    """
    PX, PY, W32, W16 = _build_consts()
    AX1 = PX @ W16  # (32,16)
    AY1 = PY @ W16
    AX = PX @ W32  # (32,32)
    AY = PY @ W32
    S = np.zeros((128, 712), np.float32)
    # L1: in rows tau*4+g (64), out rows g*32+t (128)
    for g in range(4):
        for tau in range(16):
            for t in range(32):
                S[tau * 4 + g, 0 + g * 32 + t] = AX1[t, tau]
                S[tau * 4 + g, 128 + g * 32 + t] = AY1[t, tau]
    # L2/L3: block-diag per g of AX/AY  (in g*32+s -> out g*32+t)
    for g in range(4):
        S[g * 32 : g * 32 + 32, 256 + g * 32 : 256 + g * 32 + 32] = AX.T
        S[g * 32 : g * 32 + 32, 384 + g * 32 : 384 + g * 32 + 32] = AY.T
    # L4: out rows [X-sg0 <- g1, X-sg1 <- g3, Y-sg0 <- g0, Y-sg1 <- g2]
    S[1 * 32 : 2 * 32, 512 + 0 : 512 + 32] = AX.T
    S[3 * 32 : 4 * 32, 512 + 32 : 512 + 64] = AX.T
    S[0 * 32 : 1 * 32, 512 + 64 : 512 + 96] = AY.T
    S[2 * 32 : 3 * 32, 512 + 96 : 512 + 128] = AY.T
    # L5: in [sg0(0:32), sg1(32:64)] -> out [X(0:32) <- sg1, Y(32:64) <- sg0]
    S[32:64, 640 + 0 : 640 + 32] = AX.T
    S[0:32, 640 + 32 : 640 + 64] = AY.T
    # F: C = W32 . Z5
    S[0:32, 704:712] = W32.T
    # duplicate L1 stationaries at partitions 64:128 (base-partition match for h=1)
    S[64:128, 0:256] = S[0:64, 0:256]
    return S


_STAT = None


def _stat():
    global _STAT
    if _STAT is None:
        _STAT = _build_stationaries()
    return _STAT


# ---------------------------------------------------------------------------
# host-side shard prep / final combine
# ---------------------------------------------------------------------------


def _host_prep(core, losses, imbal, phases):
    """Per-core DRAM inputs in the CC layout.

    CC partition p = kc*32 + g*8 + i_sub (kc 0..3, g 0..3, i_sub 0..7),
    free km 0..63, pair k = kc*64 + km, local iter = g*8+i_sub.
    AMP [128,256] f-blocks [L0|L1|m0|m1]; PH [128,128] f-blocks [alpha|beta].
    """
    i0 = CH * core
    kc = np.arange(4)[:, None, None]
    g = np.arange(4)[None, :, None]
    i_sub = np.arange(8)[None, None, :]
    iloc = (g * 8 + i_sub + np.zeros_like(kc)).reshape(128)
    kcf = (kc + np.zeros_like(g) + np.zeros_like(i_sub)).reshape(128)
    i = i0 + iloc  # (128,)
    ks = kcf[:, None] * 64 + np.arange(64)[None, :]  # (128, 64)
    AMP = np.empty((128, 256), np.float32)
    i2 = (2 * i)[:, None]
    AMP[:, 0:64] = losses[i2, ks]
    AMP[:, 64:128] = losses[i2 + 1, ks]
    AMP[:, 128:192] = imbal[i2, ks]
    AMP[:, 192:256] = imbal[i2 + 1, ks]
    PH = np.empty((128, 128), np.float32)
    PH[:, 0:64] = phases[i[:, None], 2 * ks]
    PH[:, 64:128] = phases[i[:, None], 2 * ks + 1]
    return AMP, PH


def _host_finish(Cs, phases_in, phases_out):
    """Combine per-core partial products and scatter into the full matrix."""
    M = np.tile(np.eye(2, dtype=np.complex128), (NPAIR, 1, 1))
    for c in range(NCORE):
        v = Cs[c].T.astype(np.float64)  # (256, 8)
        Pc = (v[:, 0::2] + 1j * v[:, 1::2]).reshape(NPAIR, 2, 2)
        M = Pc @ M
    ei = np.exp(1j * phases_in.astype(np.float64)).reshape(NPAIR, 2)
    eo = np.exp(1j * phases_out.astype(np.float64)).reshape(NPAIR, 2)
    G = (eo[:, :, None] * M * ei[:, None, :]).astype(np.complex64)
    out = np.zeros((N, N), np.complex64)
    idx = np.arange(NPAIR) * 2
    out[idx, idx] = G[:, 0, 0]
    out[idx, idx + 1] = G[:, 0, 1]
    out[idx + 1, idx] = G[:, 1, 0]
    out[idx + 1, idx + 1] = G[:, 1, 1]
    return out


# ---------------------------------------------------------------------------
# bass module
# ---------------------------------------------------------------------------

_NC = None


def _build_module():
    import concourse.bass as bass
    import concourse.bacc as bacc
    import concourse.mybir as mybir
    from concourse import tile

    f32 = mybir.dt.float32
    f32r = mybir.dt.float32r
    AF = mybir.ActivationFunctionType

    nc = bacc.Bacc("TRN2", target_bir_lowering=False, debug=False, num_devices=NCORE)
    amp_ext = nc.dram_tensor("amp", [128, 256], f32, kind="ExternalInput").ap()
    ph_ext = nc.dram_tensor("ph", [128, 128], f32, kind="ExternalInput").ap()
    stat_ext = nc.dram_tensor("stat", [128, 712], f32, kind="ExternalInput").ap()
    out_ext = nc.dram_tensor("out", [8, 256], f32, kind="ExternalOutput").ap()
    scratch = nc.dram_tensor("scratch", [64, 2048], f32r).ap()

    LN10_20 = float(np.log(10.0) / 20.0)

    with tile.TileContext(nc) as tc:
        with (
            tc.tile_pool(name="sbuf", bufs=1) as pool,
            tc.tile_pool(name="psum", bufs=1, space="PSUM") as pp,
        ):
            bhalf = pool.tile([128, 1], f32)
            bnegpi = pool.tile([128, 1], f32)
            bhalfpi = pool.tile([128, 1], f32)
            nc.gpsimd.memset(bhalf[:], 0.5)
            nc.gpsimd.memset(bnegpi[:], -float(np.pi))
            nc.gpsimd.memset(bhalfpi[:], float(np.pi / 2))

            amp = pool.tile([128, 256], f32)
            ph = pool.tile([128, 128], f32)
            statf = pool.tile([128, 712], f32)
            nc.sync.dma_start(amp[:], amp_ext[:])
            nc.sync.dma_start(ph[:], ph_ext[:])
            nc.sync.dma_start(statf[:], stat_ext[:])
            stat = pool.tile([128, 712], f32r)
            nc.vector.tensor_copy(stat[:], statf[:])

            # ---- construction (CC layout) ----
            # Only Exp and Sin ACT functions are used (2 table sets).
            # sqrt((1+-m)/2) via cubic in m (|m|<=0.05: trunc err ~2e-7), on DVE.
            expa = pool.tile([128, 128], f32)  # [a0|a1]
            nc.scalar.activation(expa[:], amp[:, 0:128], AF.Exp, scale=-LN10_20)
            R = float(1.0 / np.sqrt(2.0))
            mm = amp[:, 128:256]  # [m0|m1]
            m2 = pool.tile([128, 128], f32)
            nc.vector.tensor_mul(m2[:], mm, mm)
            ev = pool.tile([128, 128], f32)  # even part r*(1 - m^2/8)
            nc.vector.tensor_scalar(ev[:], m2[:], -R / 8.0, R,
                                    mybir.AluOpType.mult, mybir.AluOpType.add)
            t1p = pool.tile([128, 128], f32)  # r/2 + r*m^2/16
            nc.vector.tensor_scalar(t1p[:], m2[:], R / 16.0, R / 2.0,
                                    mybir.AluOpType.mult, mybir.AluOpType.add)
            ov = pool.tile([128, 128], f32)  # odd part m*(r/2 + r*m^2/16)
            nc.vector.tensor_mul(ov[:], mm, t1p[:])
            q4 = pool.tile([128, 256], f32)  # [q0p|q1p|q0m|q1m]
            nc.vector.tensor_add(q4[:, 0:128], ev[:], ov[:])
            nc.vector.tensor_sub(q4[:, 128:256], ev[:], ov[:])
            tk = pool.tile([128, 256], f32)  # [t0|t1|k0|k1]
            nc.vector.tensor_mul(tk[:, 0:128], expa[:], q4[:, 0:128])
            nc.vector.tensor_mul(tk[:, 128:256], expa[:], q4[:, 128:256])
            coef = pool.tile([128, 256], f32)  # [tt|kk|tk|kt]
            tkv = tk[:].rearrange("p (q b km) -> p q b km", q=2, b=2, km=64)
            nc.vector.tensor_mul(
                coef[:, 0:128].rearrange("p (q km) -> p q km", km=64),
                tkv[:, :, 1, :],  # [t1, k1]
                tkv[:, :, 0, :],  # [t0, k0]
            )
            nc.vector.tensor_mul(coef[:, 128:192], tk[:, 64:128], tk[:, 128:192])  # tk
            nc.vector.tensor_mul(coef[:, 192:256], tk[:, 192:256], tk[:, 0:64])  # kt
            psh = pool.tile([128, 128], f32)
            nc.vector.tensor_scalar_add(psh[:], ph[:], -float(np.pi))
            abs2 = pool.tile([128, 128], f32)
            nc.vector.scalar_tensor_tensor(abs2[:], psh[:], -1.0, psh[:],
                                           mybir.AluOpType.mult, mybir.AluOpType.max)
            trig = pool.tile([128, 256], f32)  # [ca|cb|sa|sb] (primed)
            nc.scalar.activation(trig[:, 0:128], abs2[:], AF.Sin, bias=bhalfpi[:], scale=-1.0)
            nc.scalar.activation(trig[:, 128:256], ph[:], AF.Sin, bias=bnegpi[:])

            # PRCC[p, tau*64+km] = coef[cq] * trig[taut],  tau = taut*4+cq
            prcc = pool.tile([128, 1024], f32r)
            coefv = coef[:].rearrange("p (cq km) -> p cq km", km=64).unsqueeze(1).broadcast_to((128, 4, 4, 64))
            trigv = trig[:].rearrange("p (tt km) -> p tt km", km=64).unsqueeze(2).broadcast_to((128, 4, 4, 64))
            nc.vector.tensor_mul(
                prcc[:].rearrange("p (tt cq km) -> p tt cq km", tt=4, cq=4), coefv, trigv
            )

            # ---- relayout via DRAM bounce ----
            # scratch[tau*4+g, i_sub*256+kc*64+km] = prcc[kc*32+g*8+i_sub, tau*64+km]
            # one store per kc: dst dims [(g i_sub): 32, tau: 16, km: 64]
            qeng = [nc.sync, nc.gpsimd, nc.scalar, nc.sync]
            for kc in range(4):
                srcv = prcc[kc * 32 : (kc + 1) * 32, :].rearrange(
                    "p (tau km) -> p tau km", km=64
                )
                dstv = scratch.rearrange(
                    "(tau g) (i kc2 km) -> tau g i kc2 km", g=4, kc2=4, km=64
                )[:, :, :, kc, :].rearrange("tau g i km -> (g i) tau km")
                qeng[kc].dma_start(dstv, srcv)
            pr = pool.tile([64, 2048], f32r)
            prh = pr[:].rearrange("p (i ks h) -> p i ks h", i=8, ks=2, h=128)
            sch = scratch.rearrange("p (i ks h) -> p i ks h", i=8, ks=2, h=128)
            nc.gpsimd.dma_start(prh[:, :, 0, :], sch[:, :, 0, :])
            nc.scalar.dma_start(prh[:, :, 1, :], sch[:, :, 1, :])

            # ---- tree ----
            xxp = pp.tile([128, 1024], f32, tag="psA")
            yyp = pp.tile([128, 1024], f32, tag="psB")
            prv = pr[:].rearrange("p (i ks km2) -> p i ks km2", i=8, ks=2, km2=128)
            for ks in range(2):
                mov = prv[:, :, ks, :]
                nc.tensor.matmul(
                    xxp[:, ks * 512 : (ks + 1) * 512], stat[0:64, 0:128], mov[:, 1::2, :],
                )
                nc.tensor.matmul(
                    yyp[:, ks * 512 : (ks + 1) * 512], stat[0:64, 128:256], mov[:, 0::2, :],
                )
            yys = pool.tile([128, 1024], f32)
            nc.scalar.copy(yys[:], yyp[:])
            z1 = pool.tile([128, 1024], f32r)
            nc.vector.tensor_mul(z1[:], xxp[:], yys[:])

            # L2
            z1v = z1[:].rearrange("p (h i km2) -> p h i km2", h=2, km2=128)
            xxp2 = pp.tile([128, 512], f32, tag="psC")
            yyp2 = pp.tile([128, 512], f32, tag="psD")
            nc.tensor.matmul(xxp2[:], stat[:, 256:384], z1v[:, :, 1::2, :])
            nc.tensor.matmul(yyp2[:], stat[:, 384:512], z1v[:, :, 0::2, :])
            yys2 = pool.tile([128, 512], f32)
            nc.scalar.copy(yys2[:], yyp2[:])
            z2 = pool.tile([128, 512], f32r)
            nc.vector.tensor_mul(z2[:], xxp2[:], yys2[:])

            # L3
            z2v = z2[:].rearrange("p (h i km2) -> p h i km2", h=2, km2=128)
            xxp3 = pp.tile([128, 256], f32, tag="psA")
            yyp3 = pp.tile([128, 256], f32, tag="psB")
            nc.tensor.matmul(xxp3[:], stat[:, 256:384], z2v[:, :, 1, :])
            nc.tensor.matmul(yyp3[:], stat[:, 384:512], z2v[:, :, 0, :])
            yys3 = pool.tile([128, 256], f32)
            nc.scalar.copy(yys3[:], yyp3[:])
            z3 = pool.tile([128, 256], f32r)
            nc.vector.tensor_mul(z3[:], xxp3[:], yys3[:])

            # L4 (cross-group, merged X+Y)
            p4 = pp.tile([128, 256], f32, tag="psC")
            nc.tensor.matmul(p4[:], stat[:, 512:640], z3[:])
            y4s = pool.tile([64, 256], f32)
            nc.scalar.copy(y4s[:], p4[64:128, :])
            z4 = pool.tile([64, 256], f32r)
            nc.vector.tensor_mul(z4[:], p4[0:64, :], y4s[:])

            # L5
            p5 = pp.tile([64, 256], f32, tag="psD")
            nc.tensor.matmul(p5[:], stat[0:64, 640:704], z4[:])
            y5s = pool.tile([32, 256], f32)
            nc.scalar.copy(y5s[:], p5[32:64, :])
            z5 = pool.tile([32, 256], f32r)
            nc.vector.tensor_mul(z5[:], p5[0:32, :], y5s[:])

            # final
            pc = pp.tile([8, 256], f32, tag="psA")
            nc.tensor.matmul(pc[:], stat[0:32, 704:712], z5[:])
            outt = pool.tile([8, 256], f32)
            nc.scalar.copy(outt[:], pc[:])
            nc.sync.dma_start(out_ext[:], outt[:])

    nc.finalize()
    return nc


def _get_module():
    global _NC
    if _NC is None:
        _NC = _build_module()
    return _NC


def kernel(ht_in_phase, ht_out_phase, ht_full_phases, mmi_i_losses, mmi_imbalances):
    from concourse.bass_utils import run_bass_kernel_spmd

    nc = _get_module()
    losses = np.asarray(mmi_i_losses, np.float32)
    imbal = np.asarray(mmi_imbalances, np.float32)
    phases = np.asarray(ht_full_phases, np.float32)
    stat = _stat()
    in_maps = []
    for c in range(NCORE):
        AMP, PH = _host_prep(c, losses, imbal, phases)
        in_maps.append({"amp": AMP, "ph": PH, "stat": stat})
    res = run_bass_kernel_spmd(nc, in_maps, list(range(NCORE)))
    Cs = [res.results[c]["out"] for c in range(NCORE)]
    return _host_finish(
        Cs, np.asarray(ht_in_phase, np.float32), np.asarray(ht_out_phase, np.float32)
    )



# revision 3
# speedup vs baseline: 1.8375x; 1.8375x over previous
"""Trainium2 Bass kernel for the NEUROPULS photonic-mesh transfer matrix.

The reference's crossing layers are discarded, so the full 512x512 transfer
matrix is block-diagonal with 256 independent 2x2 complex blocks:

    G_k = E_out(k) . Prod_{i=255..0} S_i(k) . E_in(k),
    S_i = B(2i+1) . diag(e^{i phi}) . B(2i),   B = [[t, i k], [i k, t]]

Layout: pairs live in PARTITIONS (pair k = 128*h + p, h in {0,1} packed in the
free dim), per-step data in the FREE dim.  Construction (amplitudes, trig,
step-matrix entries) and the binary combine tree of 2x2 complex products then
share one partition assignment -- no relayout, no PE, no PSUM: every step is a
DVE/ACT elementwise op over affine views.  Each core combines its 32 steps
down to 4 partials per pair (3 tree levels); the host chains the 8x4 partials
and scatters the blocks into the zero matrix.

Step-matrix entries (with primed trig c' = -cos, s' = -sin; S' = -S and the
sign cancels over the even number of steps):
    S00 = tt*ca - kk*cb   S01 = i*(tk*ca + kt*cb) ... where tt = t1*t0 etc.
computed as one outer product PR[cq, taut] = coef[cq] * trig[taut] plus three
paired add/sub ops over strided views.
"""

import sys

sys.path.insert(0, "/opt/trn_rl_repo")

import numpy as np

N = 512
NPAIR = 256
NCORE = 8
CH = 32  # steps per core
TWO_PI = 2.0 * np.pi

# ---------------------------------------------------------------------------
# host-side shard prep / final combine
# ---------------------------------------------------------------------------


def _host_prep(core, losses, imbal, phases):
    """Per-core DRAM input [128, 384].

    Partition p, free u = 32*h + i (h: pair-half, i: local step), pair
    k = 128*h + p, global step ig = 32*core + i.
    Free blocks of 64: [l0 | l1 | m0 | m1 | alpha | beta].
    """
    i = np.arange(CH) + CH * core  # (32,) global step
    ii = i[None, None, :]  # (1,1,32)
    kk = (np.arange(2)[None, :, None] * 128 + np.arange(128)[:, None, None])  # (128,2,1)
    INP = np.empty((128, 384), np.float32)
    INP[:, 0:64] = losses[2 * ii, kk].reshape(128, 64)
    INP[:, 64:128] = losses[2 * ii + 1, kk].reshape(128, 64)
    INP[:, 128:192] = imbal[2 * ii, kk].reshape(128, 64)
    INP[:, 192:256] = imbal[2 * ii + 1, kk].reshape(128, 64)
    INP[:, 256:320] = phases[ii, 2 * kk].reshape(128, 64)
    INP[:, 320:384] = phases[ii, 2 * kk + 1].reshape(128, 64)
    return INP


def _host_finish(Os, phases_in, phases_out):
    """Chain the per-core 4-step partials and scatter into the full matrix.

    Os[c]: (128, 64) f32, free idx = comp*8 + h*4 + jp with comp = 4r+2s+part,
    jp: which 8-step partial (later steps = higher jp = applied on the left).
    """
    M = np.tile(np.eye(2, dtype=np.complex128), (NPAIR, 1, 1))
    for c in range(NCORE):
        v = Os[c].astype(np.float64).reshape(128, 2, 2, 2, 2, 4)  # p,r,s,part,h,jp
        G = v[:, :, :, 0] + 1j * v[:, :, :, 1]  # (128, r, s, h, jp)
        G = G.transpose(0, 3, 4, 1, 2)  # (128, h, jp, 2, 2)
        Pc = G[:, :, 3] @ G[:, :, 2] @ G[:, :, 1] @ G[:, :, 0]  # (128, 2, 2, 2)
        Pk = Pc.transpose(1, 0, 2, 3).reshape(NPAIR, 2, 2)  # k = 128*h + p
        M = Pk @ M
    ei = np.exp(1j * phases_in.astype(np.float64)).reshape(NPAIR, 2)
    eo = np.exp(1j * phases_out.astype(np.float64)).reshape(NPAIR, 2)
    G = (eo[:, :, None] * M * ei[:, None, :]).astype(np.complex64)
    out = np.zeros((N, N), np.complex64)
    idx = np.arange(NPAIR) * 2
    out[idx, idx] = G[:, 0, 0]
    out[idx, idx + 1] = G[:, 0, 1]
    out[idx + 1, idx] = G[:, 1, 0]
    out[idx + 1, idx + 1] = G[:, 1, 1]
    return out


# ---------------------------------------------------------------------------
# bass module
# ---------------------------------------------------------------------------

_NC = None


def _tree_level(nc, pool, f32, S, lanes):
    """One combine level: Snext[t] = S[odd t'] @ S[even t'] (2x2 complex).

    S: tile [128, 8*lanes], comp-major (comp = 4r+2s+part), lane t = merged
    (h, j) with h-major order (preserved across levels).
    Returns Snext [128, 8*lanes//2].
    """
    L2 = lanes // 2
    P = pool.tile([128, 4 * 8 * L2], f32)  # (pa, pb, r, m, s, t2)
    Q = pool.tile([128, 2 * 8 * L2], f32)  # (qp, r, m, s, t2)
    Sn = pool.tile([128, 8 * L2], f32)

    scv = S[:].rearrange("p (c t) -> p c t", c=8)
    # A = odd steps (left factor), comp = 4r+2m+pa ; B = even, comp = 4m+2s+pb
    A = scv[:, :, 1::2].rearrange("p (r m pa) t -> p r m pa t", r=2, m=2, pa=2)
    B = scv[:, :, 0::2].rearrange("p (m s pb) t -> p m s pb t", m=2, s=2, pb=2)
    pv = P[:].rearrange("p (pa pb r m s t) -> p pa pb r m s t", pa=2, pb=2, r=2, m=2, s=2)
    for pa in range(2):
        for pb in range(2):
            op1 = A[:, :, :, pa, :].unsqueeze(3).broadcast_to((128, 2, 2, 2, L2))
            op2 = B[:, :, :, pb, :].unsqueeze(1).broadcast_to((128, 2, 2, 2, L2))
            nc.vector.tensor_mul(pv[:, pa, pb], op1, op2)
    qv = Q[:].rearrange("p (qp r m s t) -> p qp r m s t", qp=2, r=2, m=2, s=2)
    nc.vector.tensor_sub(qv[:, 0], pv[:, 0, 0], pv[:, 1, 1])
    nc.vector.tensor_add(qv[:, 1], pv[:, 0, 1], pv[:, 1, 0])
    # Snext comp = 4r+2s+qp, summed over m
    snv = (
        Sn[:]
        .rearrange("p (r s qp t) -> p r s qp t", r=2, s=2, qp=2)
        .transpose([0, 3, 1, 2, 4])
    )
    nc.vector.tensor_add(snv, qv[:, :, :, 0], qv[:, :, :, 1])
    return Sn


def _build_module():
    import concourse.bass as bass
    import concourse.bacc as bacc
    import concourse.mybir as mybir
    from concourse import tile

    f32 = mybir.dt.float32
    AF = mybir.ActivationFunctionType

    nc = bacc.Bacc("TRN2", target_bir_lowering=False, debug=False, num_devices=NCORE)
    inp_ext = nc.dram_tensor("inp", [128, 384], f32, kind="ExternalInput").ap()
    out_ext = nc.dram_tensor("out", [128, 64], f32, kind="ExternalOutput").ap()

    LN10_20 = float(np.log(10.0) / 20.0)
    R = float(1.0 / np.sqrt(2.0))

    with tile.TileContext(nc) as tc:
        with tc.tile_pool(name="sbuf", bufs=1) as pool:
            bnegpi = pool.tile([128, 1], f32)
            bhalfpi = pool.tile([128, 1], f32)
            dmy = pool.tile([1, 2], f32)
            dmy2 = pool.tile([1, 2], f32)
            nc.gpsimd.memset(bnegpi[:], -float(np.pi))
            nc.gpsimd.memset(bhalfpi[:], float(np.pi / 2))
            nc.gpsimd.memset(dmy[:], 0.0)

            inp = pool.tile([128, 384], f32)
            # split input DMA across the three DMA-capable queues; m-block
            # first (it heads the DVE chain)
            nc.sync.dma_start(inp[:, 128:256], inp_ext[:, 128:256])
            nc.scalar.dma_start(inp[:, 0:128], inp_ext[:, 0:128])
            nc.gpsimd.dma_start(inp[:, 256:384], inp_ext[:, 256:384])

            # preload both ACT table sets under the input-DMA latency
            nc.scalar.activation(dmy2[:], dmy[:], AF.Exp)
            nc.scalar.activation(dmy2[:], dmy[:], AF.Sin)

            # ---- construction ----
            mm = inp[:, 128:256]  # [m0|m1]
            m2 = pool.tile([128, 128], f32)
            nc.vector.tensor_mul(m2[:], mm, mm)
            ev = pool.tile([128, 128], f32)  # even part r*(1 - m^2/8)
            nc.vector.tensor_scalar(ev[:], m2[:], -R / 8.0, R,
                                    mybir.AluOpType.mult, mybir.AluOpType.add)
            t1p = pool.tile([128, 128], f32)  # r/2 + r*m^2/16
            nc.vector.tensor_scalar(t1p[:], m2[:], R / 16.0, R / 2.0,
                                    mybir.AluOpType.mult, mybir.AluOpType.add)
            ov = pool.tile([128, 128], f32)  # odd part m*(r/2 + r*m^2/16)
            nc.vector.tensor_mul(ov[:], mm, t1p[:])
            q4 = pool.tile([128, 256], f32)  # [q0p|q1p|q0m|q1m]
            nc.vector.tensor_add(q4[:, 0:128], ev[:], ov[:])
            nc.vector.tensor_sub(q4[:, 128:256], ev[:], ov[:])
            psh = pool.tile([128, 128], f32)
            nc.vector.tensor_scalar_add(psh[:], inp[:, 256:384], -float(np.pi))
            abs2 = pool.tile([128, 128], f32)
            nc.vector.scalar_tensor_tensor(abs2[:], psh[:], -1.0, psh[:],
                                           mybir.AluOpType.mult, mybir.AluOpType.max)

            expa = pool.tile([128, 128], f32)  # [a0|a1]
            nc.scalar.activation(expa[:], inp[:, 0:128], AF.Exp, scale=-LN10_20)
            trig = pool.tile([128, 256], f32)  # [ca'|cb'|sa'|sb'] (primed)
            nc.scalar.activation(trig[:, 128:256], inp[:, 256:384], AF.Sin, bias=bnegpi[:])
            nc.scalar.activation(trig[:, 0:128], abs2[:], AF.Sin, bias=bhalfpi[:], scale=-1.0)

            tk = pool.tile([128, 256], f32)  # [t0|t1|k0|k1]
            tkv = tk[:].rearrange("p (x m u) -> p x m u", x=2, m=2)
            eop = expa[:].rearrange("p (m u) -> p m u", m=2).unsqueeze(1).broadcast_to((128, 2, 2, 64))
            q4v = q4[:].rearrange("p (x m u) -> p x m u", x=2, m=2)
            nc.vector.tensor_mul(tkv, eop, q4v)
            # coef [tt|tk|kt|kk]: (c1,c0) -> tk1[c1] * tk0[c0]
            coef = pool.tile([128, 256], f32)
            cfv = coef[:].rearrange("p (c1 c0 u) -> p c1 c0 u", c1=2, c0=2)
            op1 = tk[:].rearrange("p (x m u) -> p x m u", x=2, m=2)[:, :, 1, :].unsqueeze(2).broadcast_to((128, 2, 2, 64))
            op2 = tk[:].rearrange("p (x m u) -> p x m u", x=2, m=2)[:, :, 0, :].unsqueeze(1).broadcast_to((128, 2, 2, 64))
            nc.vector.tensor_mul(cfv, op1, op2)
            # PR[cq, taut] = coef[cq] * trig[taut]
            pr = pool.tile([128, 1024], f32)
            prv = pr[:].rearrange("p (cq t u) -> p cq t u", cq=4, t=4)
            cop = coef[:].rearrange("p (cq u) -> p cq u", cq=4).unsqueeze(2).broadcast_to((128, 4, 4, 64))
            top = trig[:].rearrange("p (t u) -> p t u", t=4).unsqueeze(1).broadcast_to((128, 4, 4, 64))
            nc.vector.tensor_mul(prv, cop, top)

            # ---- combine into S' (comp = 4r+2s+part, primed sign) ----
            S = pool.tile([128, 512], f32)
            sv = S[:].rearrange("p (c u) -> p c u", c=8)
            # re comps {S00re:0, S11re:6}: PR[tt, ca/cb] - PR[kk, cb/ca]
            nc.vector.tensor_sub(sv[:, 0:8:6], prv[:, 0, 0:2], prv[:, 3, 0:2][:, ::-1])
            # im comps {S00im:1, S11im:7}: PR[tt, sa/sb] - PR[kk, sb/sa]
            nc.vector.tensor_sub(sv[:, 1:8:6], prv[:, 0, 2:4], prv[:, 3, 2:4][:, ::-1])
            # {S01im:3, S10im:5}: PR[tk/kt, ca] + PR[kt/tk, cb]
            nc.vector.tensor_add(sv[:, 3:6:2], prv[:, 1:3, 0], prv[:, 1:3, 1][:, ::-1])
            # {S01re:2, S10re:4}: -(PR[tk/kt, sa] + PR[kt/tk, sb])
            nc.vector.scalar_tensor_tensor(sv[:, 2:5:2], prv[:, 1:3, 2], -1.0,
                                           prv[:, 1:3, 3][:, ::-1],
                                           mybir.AluOpType.mult, mybir.AluOpType.subtract)

            # ---- combine tree: 32 steps -> 4 partials ----
            S2 = _tree_level(nc, pool, f32, S, 64)
            S3 = _tree_level(nc, pool, f32, S2, 32)
            O = _tree_level(nc, pool, f32, S3, 16)
            nc.sync.dma_start(out_ext[:], O[:])

    nc.finalize()
    return nc


def _get_module():
    global _NC
    if _NC is None:
        _NC = _build_module()
    return _NC


def kernel(ht_in_phase, ht_out_phase, ht_full_phases, mmi_i_losses, mmi_imbalances):
    from concourse.bass_utils import run_bass_kernel_spmd

    nc = _get_module()
    losses = np.asarray(mmi_i_losses, np.float32)
    imbal = np.asarray(mmi_imbalances, np.float32)
    phases = np.asarray(ht_full_phases, np.float32)
    in_maps = [{"inp": _host_prep(c, losses, imbal, phases)} for c in range(NCORE)]
    res = run_bass_kernel_spmd(nc, in_maps, list(range(NCORE)))
    Os = [res.results[c]["out"] for c in range(NCORE)]
    return _host_finish(
        Os, np.asarray(ht_in_phase, np.float32), np.asarray(ht_out_phase, np.float32)
    )


# revision 4
# speedup vs baseline: 2.4704x; 1.3444x over previous
"""Trainium2 Bass kernel for the NEUROPULS photonic-mesh transfer matrix.

The reference's crossing layers are discarded, so the full 512x512 transfer
matrix is block-diagonal with 256 independent 2x2 complex blocks:

    G_k = E_out(k) . Prod_{i=255..0} S_i(k) . E_in(k),
    S_i = B(2i+1) . diag(e^{i phi}) . B(2i),   B = [[t, i k], [i k, t]]

Layout: pairs live in PARTITIONS (pair k = 128*h + p, h packed in the free
dim), per-step data in the FREE dim.  Construction (amplitudes, trig, step
matrix entries) and the binary combine tree of 2x2 complex products then share
one partition assignment -- no relayout, no PE, no PSUM: everything is a
DVE/ACT/Pool elementwise op over affine views.

Steps are stored in bit-reversed lane order, so at every tree level the left
(odd) and right (even) factors are contiguous blocks and the last view dim
stays packed -- enabling the DVE 2x 16-bit mode for the fp16 combine tree.
The amplitude exponential 10^(-l/20) is a DVE cubic (|x| <= 0.058), so only
the Sin activation-table set is ever loaded (one 1283 ns load, hoisted under
the input-DMA latency).

Step-matrix entries (with primed trig c' = -cos, s' = -sin; S' = -S and the
sign cancels over the even number of steps):
    S00 = tt*ca - kk*cb, S01 = i*(tk*ca + kt*cb), ...  (tt = t1*t0 etc.)
computed as one outer product PR[cq, taut] = coef[cq] * trig[taut] plus four
paired add/sub ops over strided views.  Each core combines its 32 steps down
to 4 partials per pair; the host chains the 8x4 partials and scatters the
2x2 blocks into the zero matrix.
"""

import sys

sys.path.insert(0, "/opt/trn_rl_repo")

import numpy as np

N = 512
NPAIR = 256
NCORE = 8
CH = 32  # steps per core
TWO_PI = 2.0 * np.pi

_BR5 = np.array([int(f"{i:05b}"[::-1], 2) for i in range(32)])  # bit-reverse
_BR2 = np.array([0, 2, 1, 3])

# ---------------------------------------------------------------------------
# host-side shard prep / final combine
# ---------------------------------------------------------------------------


def _host_prep(core, losses, imbal, phases):
    """Per-core DRAM input [128, 384].

    Partition p, free u = 32*h + bitrev5(i) (h: pair-half, i: local step),
    pair k = 128*h + p, global step ig = 32*core + i.
    Free blocks of 64: [l0 | l1 | m0 | m1 | alpha | beta].
    """
    i = np.arange(CH) + CH * core  # (32,) global step
    ii = i[None, None, :]  # (1,1,32)
    kk = (np.arange(2)[None, :, None] * 128 + np.arange(128)[:, None, None])  # (128,2,1)

    def pack(block):  # (128, 2, 32) -> (128, 64) with bit-reversed step order
        out = np.empty((128, 2, 32), np.float32)
        out[:, :, _BR5] = block
        return out.reshape(128, 64)

    INP = np.empty((128, 384), np.float32)
    INP[:, 0:64] = pack(losses[2 * ii, kk])
    INP[:, 64:128] = pack(losses[2 * ii + 1, kk])
    INP[:, 128:192] = pack(imbal[2 * ii, kk])
    INP[:, 192:256] = pack(imbal[2 * ii + 1, kk])
    INP[:, 256:320] = pack(phases[ii, 2 * kk])
    INP[:, 320:384] = pack(phases[ii, 2 * kk + 1])
    return INP


def _host_finish(Os, phases_in, phases_out):
    """Chain the per-core 4-step partials and scatter into the full matrix.

    Os[c]: (128, 64) fp16, free idx = comp*8 + h*4 + pos with comp = 4r+2s+part
    and pos = bitrev2(jp); partial jp covers steps [8jp, 8jp+8) (later steps =
    applied on the left).
    """
    M = np.tile(np.eye(2, dtype=np.complex128), (NPAIR, 1, 1))
    for c in range(NCORE):
        v = Os[c].astype(np.float64).reshape(128, 2, 2, 2, 2, 4)  # p,r,s,part,h,pos
        v = v[..., _BR2]  # jp order
        G = v[:, :, :, 0] + 1j * v[:, :, :, 1]  # (128, r, s, h, jp)
        G = G.transpose(0, 3, 4, 1, 2)  # (128, h, jp, 2, 2)
        Pc = G[:, :, 3] @ G[:, :, 2] @ G[:, :, 1] @ G[:, :, 0]  # (128, 2, 2, 2)
        Pk = Pc.transpose(1, 0, 2, 3).reshape(NPAIR, 2, 2)  # k = 128*h + p
        M = Pk @ M
    ei = np.exp(1j * phases_in.astype(np.float64)).reshape(NPAIR, 2)
    eo = np.exp(1j * phases_out.astype(np.float64)).reshape(NPAIR, 2)
    G = (eo[:, :, None] * M * ei[:, None, :]).astype(np.complex64)
    out = np.zeros((N, N), np.complex64)
    idx = np.arange(NPAIR) * 2
    out[idx, idx] = G[:, 0, 0]
    out[idx, idx + 1] = G[:, 0, 1]
    out[idx + 1, idx] = G[:, 1, 0]
    out[idx + 1, idx + 1] = G[:, 1, 1]
    return out


# ---------------------------------------------------------------------------
# bass module
# ---------------------------------------------------------------------------

_NC = None


def _tree_level(nc, pool, f16, S, lanes):
    """One combine level: Snext[j] = S[odd] @ S[even] (2x2 complex, fp16).

    S: tile [128, 8*lanes], free = (comp, h, half, pos): comp = 4r+2s+part,
    half: even/odd step (bit-reversed order), pos: packed position.
    Returns Snext [128, 8*lanes//2] with free = (comp, h, pos).
    """
    T = lanes // 4  # packed positions per (h, half)
    P = pool.tile([128, 4 * 8 * 2 * T], f16)  # (pa, pb, r, m, s, h, t)
    Q = pool.tile([128, 2 * 8 * 2 * T], f16)  # (qp, r, m, s, h, t)
    Sn = pool.tile([128, 8 * 2 * T], f16)

    scv = S[:].rearrange("p (c h half t) -> p c h half t", c=8, h=2, half=2)
    # A = odd steps (left factor), comp = 4r+2m+pa ; B = even, comp = 4m+2s+pb
    A = scv[:, :, :, 1].rearrange("p (r m pa) h t -> p r m pa h t", r=2, m=2, pa=2)
    B = scv[:, :, :, 0].rearrange("p (m s pb) h t -> p m s pb h t", m=2, s=2, pb=2)
    pv = P[:].rearrange(
        "p (pa pb r m s h t) -> p pa pb r m s h t", pa=2, pb=2, r=2, m=2, s=2, h=2
    )
    for pa in range(2):
        for pb in range(2):
            op1 = A[:, :, :, pa].unsqueeze(3).broadcast_to((128, 2, 2, 2, 2, T))
            op2 = B[:, :, :, pb].unsqueeze(1).broadcast_to((128, 2, 2, 2, 2, T))
            nc.vector.tensor_mul(pv[:, pa, pb], op1, op2)
    qv = Q[:].rearrange("p (qp r m s h t) -> p qp r m s h t", qp=2, r=2, m=2, s=2, h=2)
    nc.vector.tensor_sub(qv[:, 0], pv[:, 0, 0], pv[:, 1, 1])
    nc.vector.tensor_add(qv[:, 1], pv[:, 0, 1], pv[:, 1, 0])
    # Snext comp = 4r+2s+qp, summed over m
    snv = (
        Sn[:]
        .rearrange("p (r s qp h t) -> p r s qp h t", r=2, s=2, qp=2, h=2)
        .transpose([0, 3, 1, 2, 4, 5])
    )
    nc.vector.tensor_add(snv, qv[:, :, :, 0], qv[:, :, :, 1])
    return Sn


def _build_module():
    import concourse.bass as bass
    import concourse.bacc as bacc
    import concourse.mybir as mybir
    from concourse import tile

    f32 = mybir.dt.float32
    f16 = mybir.dt.float16
    AF = mybir.ActivationFunctionType
    ALU = mybir.AluOpType

    nc = bacc.Bacc("TRN2", target_bir_lowering=False, debug=False, num_devices=NCORE)
    inp_ext = nc.dram_tensor("inp", [128, 384], f32, kind="ExternalInput").ap()
    out_ext = nc.dram_tensor("out", [128, 64], f16, kind="ExternalOutput").ap()

    C10 = float(np.log(10.0) / 20.0)  # a = exp(-C10 * l)
    R = float(1.0 / np.sqrt(2.0))

    with tile.TileContext(nc) as tc:
        with tc.tile_pool(name="sbuf", bufs=1) as pool:
            bnegpi = pool.tile([128, 1], f32)
            bhalfpi = pool.tile([128, 1], f32)
            nc.gpsimd.memset(bnegpi[:], -float(np.pi))
            nc.gpsimd.memset(bhalfpi[:], float(np.pi / 2))

            inp = pool.tile([128, 384], f32)
            # m-block first on SP (it heads the DVE chain), l second on SP,
            # alpha/beta on the Pool SWDGE queue
            nc.sync.dma_start(inp[:, 128:256], inp_ext[:, 128:256])
            nc.sync.dma_start(inp[:, 0:128], inp_ext[:, 0:128])
            nc.gpsimd.dma_start(inp[:, 256:384], inp_ext[:, 256:384])

            # ---- construction ----
            mm = inp[:, 128:256]  # [m0|m1]
            m2 = pool.tile([128, 128], f32)
            nc.vector.tensor_mul(m2[:], mm, mm)
            ev = pool.tile([128, 128], f32)  # even part r*(1 - m^2/8)
            nc.vector.tensor_scalar(ev[:], m2[:], -R / 8.0, R, ALU.mult, ALU.add)
            t1p = pool.tile([128, 128], f32)  # r/2 + r*m^2/16
            nc.vector.tensor_scalar(t1p[:], m2[:], R / 16.0, R / 2.0, ALU.mult, ALU.add)
            ov = pool.tile([128, 128], f32)  # odd part m*(r/2 + r*m^2/16)
            nc.vector.tensor_mul(ov[:], mm, t1p[:])
            q4 = pool.tile([128, 256], f32)  # [q0p|q1p|q0m|q1m]
            nc.vector.tensor_add(q4[:, 0:128], ev[:], ov[:])
            nc.vector.tensor_sub(q4[:, 128:256], ev[:], ov[:])

            # amplitude exp as cubic: e = (1 + x) + l^2*(C^2/2 - C^3 l/6),
            # x = -C*l, |x| <= 0.058 (err < 5e-7)
            ll = inp[:, 0:128]  # [l0|l1]
            pA = pool.tile([128, 128], f32)
            nc.vector.tensor_scalar(pA[:], ll, -(C10**3) / 6.0, C10**2 / 2.0,
                                    ALU.mult, ALU.add)
            pX = pool.tile([128, 128], f32)
            nc.vector.tensor_scalar(pX[:], ll, -C10, 1.0, ALU.mult, ALU.add)
            pL2 = pool.tile([128, 128], f32)
            nc.vector.tensor_mul(pL2[:], ll, ll)
            pP = pool.tile([128, 128], f32)
            nc.vector.tensor_mul(pP[:], pL2[:], pA[:])
            expa = pool.tile([128, 128], f32)  # [a0|a1]
            nc.vector.tensor_add(expa[:], pX[:], pP[:])

            # phase prep on Pool (ACT needs |ph - pi| for the cos recipe)
            psh = pool.tile([128, 128], f32)
            nc.gpsimd.tensor_scalar_add(psh[:], inp[:, 256:384], -float(np.pi))
            abs2 = pool.tile([128, 128], f32)
            nc.gpsimd.scalar_tensor_tensor(abs2[:], psh[:], -1.0, psh[:],
                                           ALU.mult, ALU.max)
            trig = pool.tile([128, 256], f32)  # [ca'|cb'|sa'|sb'] (primed)
            nc.scalar.activation(trig[:, 128:256], inp[:, 256:384], AF.Sin, bias=bnegpi[:])
            nc.scalar.activation(trig[:, 0:128], abs2[:], AF.Sin, bias=bhalfpi[:], scale=-1.0)

            tk = pool.tile([128, 256], f32)  # [t0|t1|k0|k1]
            tkv = tk[:].rearrange("p (x m u) -> p x m u", x=2, m=2)
            eop = expa[:].rearrange("p (m u) -> p m u", m=2).unsqueeze(1).broadcast_to((128, 2, 2, 64))
            q4v = q4[:].rearrange("p (x m u) -> p x m u", x=2, m=2)
            nc.vector.tensor_mul(tkv, eop, q4v)
            # coef [tt|tk|kt|kk]: (c1,c0) -> tk1[c1] * tk0[c0]
            coef = pool.tile([128, 256], f32)
            cfv = coef[:].rearrange("p (c1 c0 u) -> p c1 c0 u", c1=2, c0=2)
            op1 = tk[:].rearrange("p (x m u) -> p x m u", x=2, m=2)[:, :, 1, :].unsqueeze(2).broadcast_to((128, 2, 2, 64))
            op2 = tk[:].rearrange("p (x m u) -> p x m u", x=2, m=2)[:, :, 0, :].unsqueeze(1).broadcast_to((128, 2, 2, 64))
            nc.vector.tensor_mul(cfv, op1, op2)

            # PR[cq, taut] = coef[cq] * trig[taut]; kk row (cq=3) on Pool
            pr = pool.tile([128, 768], f32)
            prkk = pool.tile([128, 256], f32)
            prv = pr[:].rearrange("p (cq t u) -> p cq t u", cq=3, t=4)
            cop = coef[:, 0:192].rearrange("p (cq u) -> p cq u", cq=3).unsqueeze(2).broadcast_to((128, 3, 4, 64))
            top = trig[:].rearrange("p (t u) -> p t u", t=4).unsqueeze(1).broadcast_to((128, 3, 4, 64))
            nc.vector.tensor_mul(prv, cop, top)
            pkv = prkk[:].rearrange("p (t u) -> p t u", t=4)
            kop = coef[:, 192:256].unsqueeze(1).broadcast_to((128, 4, 64))
            top2 = trig[:].rearrange("p (t u) -> p t u", t=4)
            nc.gpsimd.tensor_mul(pkv, kop, top2)

            # ---- combine into S' (fp16, comp = 4r+2s+part) ----
            S = pool.tile([128, 512], f16)
            sv = S[:].rearrange("p (c u) -> p c u", c=8)
            # re {S00re:0, S11re:6}: PR[tt, ca/cb] - PR[kk, cb/ca]
            nc.vector.tensor_sub(sv[:, 0:8:6], prv[:, 0, 0:2], pkv[:, 0:2][:, ::-1])
            # im {S00im:1, S11im:7}: PR[tt, sa/sb] - PR[kk, sb/sa]
            nc.vector.tensor_sub(sv[:, 1:8:6], prv[:, 0, 2:4], pkv[:, 2:4][:, ::-1])
            # {S01im:3, S10im:5}: PR[tk/kt, ca] + PR[kt/tk, cb]
            nc.vector.tensor_add(sv[:, 3:6:2], prv[:, 1:3, 0], prv[:, 1:3, 1][:, ::-1])
            # {S01re:2, S10re:4}: -(PR[tk/kt, sa] + PR[kt/tk, sb])
            nc.vector.scalar_tensor_tensor(sv[:, 2:5:2], prv[:, 1:3, 2], -1.0,
                                           prv[:, 1:3, 3][:, ::-1],
                                           ALU.mult, ALU.subtract)

            # ---- fp16 combine tree: 32 steps -> 4 partials ----
            S2 = _tree_level(nc, pool, f16, S, 64)
            S3 = _tree_level(nc, pool, f16, S2, 32)
            O = _tree_level(nc, pool, f16, S3, 16)
            nc.sync.dma_start(out_ext[:], O[:])

    nc.finalize()
    return nc


def _get_module():
    global _NC
    if _NC is None:
        _NC = _build_module()
    return _NC


def kernel(ht_in_phase, ht_out_phase, ht_full_phases, mmi_i_losses, mmi_imbalances):
    from concourse.bass_utils import run_bass_kernel_spmd

    nc = _get_module()
    losses = np.asarray(mmi_i_losses, np.float32)
    imbal = np.asarray(mmi_imbalances, np.float32)
    phases = np.asarray(ht_full_phases, np.float32)
    in_maps = [{"inp": _host_prep(c, losses, imbal, phases)} for c in range(NCORE)]
    res = run_bass_kernel_spmd(nc, in_maps, list(range(NCORE)))
    Os = [res.results[c]["out"] for c in range(NCORE)]
    return _host_finish(
        Os, np.asarray(ht_in_phase, np.float32), np.asarray(ht_out_phase, np.float32)
    )
